# revision 1
# baseline (speedup 1.0000x reference)
"""GCN message-passing Bass kernel for TRN2 (8 cores).

Math: delta = segment_sum(w_e * x[src_e]) @ W^T   (linearity: transform after aggregate)

Sharding: targets split across 8 cores (12500 each). Per core, targets are
degree-sorted and grouped into 128-target blocks; each block-j target p has
D_j padded edge slots. One indirect DMA gathers x rows for a whole block:
out[p, d, :] = x[idx[p, d], :]  (pad slots point at row 0, weight 0).
DVE multiplies by per-slot weights (broadcast AP) and reduces over slots.
PE transposes agg and applies W^T; indirect DMA scatters final rows.
"""

import math
from contextlib import ExitStack

import numpy as np

import concourse.bass as bass
import concourse.bacc as bacc
import concourse.mybir as mybir
import concourse.tile as tile
from concourse.bass import IndirectOffsetOnAxis
from concourse.bass_utils import run_bass_kernel_spmd

P = 128
N_CORES = 8
F32 = mybir.dt.float32
I32 = mybir.dt.int32


def preprocess(source, target, edge_weights, n_nodes, n_cores=N_CORES):
    """Build per-core gather/weight/target-id arrays and the shared block schedule.

    Returns dict with:
      d_sched: list[int] per-block slot count (same for all cores)
      per_core: list of dicts with idx_all [128,S] i32, w_all [128,S] f32,
                tgt_all [128,nblk] i32
      nt: targets per core, nblk: blocks per core
    """
    source = np.asarray(source).astype(np.int64)
    target = np.asarray(target).astype(np.int64)
    edge_weights = np.asarray(edge_weights).astype(np.float32)
    nt = n_nodes // n_cores
    assert nt * n_cores == n_nodes
    nblk = math.ceil(nt / P)

    cores = []
    for k in range(n_cores):
        lo, hi = k * nt, (k + 1) * nt
        m = (target >= lo) & (target < hi)
        src_k = source[m]
        w_k = edge_weights[m]
        tl_k = target[m] - lo  # local target ids

        deg = np.bincount(tl_k, minlength=nt)
        perm = np.argsort(deg, kind="stable")  # local ids, degree-ascending
        # block j holds targets perm[j*128:(j+1)*128]; slot count = max degree in block
        deg_sorted = deg[perm]
        d_k = [int(deg_sorted[j * P : (j + 1) * P].max()) if j * P < nt else 0
               for j in range(nblk)]
        cores.append(dict(src=src_k, w=w_k, tl=tl_k, deg=deg, perm=perm, d_k=d_k))

    d_sched = [max(c["d_k"][j] for c in cores) for j in range(nblk)]
    S = sum(d_sched)
    offs = np.concatenate([[0], np.cumsum(d_sched)]).astype(np.int64)

    per_core = []
    for k in range(n_cores):
        c = cores[k]
        # CSR by local target id
        order = np.argsort(c["tl"], kind="stable")
        src_s, w_s = c["src"][order], c["w"][order]
        starts = np.concatenate([[0], np.cumsum(c["deg"])]).astype(np.int64)

        # Paired gather: idx addresses row-pairs of x viewed as [n/2, 128];
        # each slot expands to two weight columns (even/odd row of the pair).
        idx_all = np.zeros((P, S), dtype=np.int32)  # pad -> pair 0 (weights 0)
        w_all = np.zeros((P, 2 * S), dtype=np.float32)
        tgt_all = np.full((P, nblk), 1 << 20, dtype=np.int32)  # pad -> OOB skip
        perm = c["perm"]
        for j in range(nblk):
            o = offs[j]
            blk = perm[j * P : (j + 1) * P]
            for p, t in enumerate(blk):
                s0, d = starts[t], c["deg"][t]
                if d:
                    sl = src_s[s0 : s0 + d]
                    idx_all[p, o : o + d] = sl >> 1
                    w_all[p, 2 * o + 2 * np.arange(d) + (sl & 1)] = w_s[s0 : s0 + d]
                tgt_all[p, j] = t
        per_core.append(dict(idx_all=idx_all, w_all=w_all, tgt_all=tgt_all))

    return dict(d_sched=d_sched, S=S, per_core=per_core, nt=nt, nblk=nblk)


def build_nc(d_sched, S, n_nodes, nt, nblk, d_feat=64, bufs=3):
    nc = bacc.Bacc("TRN2", target_bir_lowering=False, debug=False)
    D = d_feat
    x_t = nc.dram_tensor("x", [n_nodes // 2, 2 * D], F32, kind="ExternalInput")
    wt_t = nc.dram_tensor("wT", [D, D], F32, kind="ExternalInput")
    idx_t = nc.dram_tensor("idx", [P, S], I32, kind="ExternalInput")
    wgt_t = nc.dram_tensor("wgt", [P, 2 * S], F32, kind="ExternalInput")
    tgt_t = nc.dram_tensor("tgt", [P, nblk], I32, kind="ExternalInput")
    eye_t = nc.dram_tensor("eye", [P, P], F32, kind="ExternalInput")
    out_t = nc.dram_tensor("out", [nt, D], F32, kind="ExternalOutput")

    with tile.TileContext(nc) as tc, ExitStack() as ctx:
        const = ctx.enter_context(tc.tile_pool(name="const", bufs=1))
        gpool = ctx.enter_context(tc.tile_pool(name="gather", bufs=bufs))
        mpool = ctx.enter_context(tc.tile_pool(name="msg", bufs=bufs))
        apool = ctx.enter_context(tc.tile_pool(name="agg", bufs=bufs))
        tpool = ctx.enter_context(tc.tile_pool(name="aggT", bufs=bufs))
        dpool = ctx.enter_context(tc.tile_pool(name="delta", bufs=bufs))
        psum = ctx.enter_context(tc.tile_pool(name="psum", bufs=4, space="PSUM"))

        ident = const.tile([P, P], F32)
        nc.sync.dma_start(out=ident[:], in_=eye_t.ap())
        wt_sb = const.tile([D, D], F32)
        nc.sync.dma_start(out=wt_sb[:], in_=wt_t.ap())
        idx_sb = const.tile([P, S], I32)
        nc.sync.dma_start(out=idx_sb[:], in_=idx_t.ap())
        wgt_sb = const.tile([P, 2 * S], F32)
        nc.sync.dma_start(out=wgt_sb[:], in_=wgt_t.ap())
        tgt_sb = const.tile([P, nblk], I32)
        nc.sync.dma_start(out=tgt_sb[:], in_=tgt_t.ap())
        bounds_reg = nc.gpsimd.to_reg(nt - 1)

        # Prime engines on the upfront loads so per-block instructions carry
        # at most one sync wait each (SEQ instruction structs encode one).
        prime = const.tile([P, 1], F32)
        nc.vector.tensor_copy(out=prime[:], in_=wgt_sb[:, :1])
        prime_ps = psum.tile([P, P], F32, tag="tp")
        nc.tensor.transpose(out=prime_ps[:], in_=ident[:], identity=ident[:])
        nc.tensor.transpose(out=prime_ps[:D, :D], in_=wt_sb[:], identity=ident[:D, :D])

        off = 0
        for j in range(nblk):
            dj = d_sched[j]
            agg = apool.tile([P, D], F32, tag="agg")
            if dj > 0:
                g = gpool.tile([P, dj * 2 * D], F32, tag="g")
                for dd in range(dj):
                    nc.gpsimd.indirect_dma_start(
                        out=g[:, dd * 2 * D : (dd + 1) * 2 * D],
                        out_offset=None,
                        in_=x_t.ap(),
                        in_offset=IndirectOffsetOnAxis(
                            ap=idx_sb[:, off + dd : off + dd + 1], axis=0
                        ),
                    )
                msg = mpool.tile([P, dj * 2 * D], F32, tag="m")
                nc.vector.tensor_tensor(
                    out=msg[:].rearrange("p (d o) -> p d o", o=D),
                    in0=g[:].rearrange("p (d o) -> p d o", o=D),
                    in1=wgt_sb[:, 2 * off : 2 * (off + dj)].to_broadcast(
                        [P, 2 * dj, D]
                    ),
                    op=mybir.AluOpType.mult,
                )
                nc.vector.tensor_reduce(
                    out=agg[:],
                    in_=msg[:].rearrange("p (d o) -> p o d", o=D),
                    axis=mybir.AxisListType.X,
                    op=mybir.AluOpType.add,
                )
            else:
                nc.vector.memset(agg[:], 0.0)

            agg_ps = psum.tile([D, P], F32, tag="tp")
            nc.tensor.transpose(out=agg_ps[:], in_=agg[:], identity=ident[:])
            agg_tr = tpool.tile([D, P], F32, tag="aT")
            nc.vector.tensor_copy(out=agg_tr[:], in_=agg_ps[:])

            d_ps = psum.tile([P, D], F32, tag="mm")
            nc.tensor.matmul(out=d_ps[:], lhsT=agg_tr[:], rhs=wt_sb[:], start=True, stop=True)
            d_sb = dpool.tile([P, D], F32, tag="d")
            nc.vector.tensor_copy(out=d_sb[:], in_=d_ps[:])

            nc.gpsimd.indirect_dma_start(
                out=out_t.ap(),
                out_offset=IndirectOffsetOnAxis(ap=tgt_sb[:, j : j + 1], axis=0),
                in_=d_sb[:],
                in_offset=None,
                bounds_check=bounds_reg,
                oob_is_err=False,
            )
            off += dj
    nc.compile()
    return nc


def run_gcn(x, W, edge_weights, source, target, num_nodes, trace=False, bufs=3):
    """Full-input host entry: preprocess, build, run on 8 cores, assemble output."""
    n_nodes = int(num_nodes)
    pp = preprocess(source, target, edge_weights, n_nodes)
    nc = build_nc(pp["d_sched"], pp["S"], n_nodes, pp["nt"], pp["nblk"],
                  d_feat=x.shape[1], bufs=bufs)
    x_np = np.ascontiguousarray(np.asarray(x), dtype=np.float32).reshape(
        n_nodes // 2, 2 * x.shape[1]
    )
    wt_np = np.ascontiguousarray(np.asarray(W).T, dtype=np.float32)
    in_maps = []
    for k in range(N_CORES):
        pc = pp["per_core"][k]
        in_maps.append({
            "x": x_np, "wT": wt_np, "eye": np.eye(P, dtype=np.float32),
            "idx": pc["idx_all"], "wgt": pc["w_all"], "tgt": pc["tgt_all"],
        })
    res = run_bass_kernel_spmd(nc, in_maps, core_ids=list(range(N_CORES)), trace=trace)
    out = np.concatenate([res.results[k]["out"] for k in range(N_CORES)], axis=0)
    return out, res


def kernel(**inputs) -> np.ndarray:
    """Harness entry: full unsharded inputs -> full (num_nodes, 64) output."""
    out, _ = run_gcn(
        np.asarray(inputs["x"]),
        np.asarray(inputs["W"]),
        np.asarray(inputs["edge_weights"]),
        np.asarray(inputs["source"]),
        np.asarray(inputs["target"]),
        int(inputs["num_nodes"]),
        trace=False,
    )
    return out



# revision 28
# speedup vs baseline: 11.2952x; 11.2952x over previous
"""GCN message-passing Bass kernel for TRN2 (8 cores).

Math: delta = segment_sum(w_e * x[src_e]) @ W^T  (linearity: transform after
aggregate).

Sharding: 4 source-quarters x 2 target-halves = 8 cores. Each core handles
edges whose source lies in its 25k-row quarter of x and whose target lies in
its 50k half; it produces a partial (transposed) delta for its half. The host
sums the 4 quarter-partials per half (the all-reduce/unshard step) -- valid
because delta = (sum_q agg_q) @ W^T is linear.

Per core, targets are degree-sorted (per-core degrees) into 128-target
blocks; block j has d_j padded edge slots. Most slots are fetched by ANT
dma_gathers (int16 indices into the 25k-row quarter -- this is why the
source dimension is quartered), amortizing the ~1us SWDGE descriptor-gen
cost that dominated the per-slot indirect-DMA baseline; a small set of
1-2 slot blocks instead uses per-slot indirect DMAs against a bf16 copy of
the quarter (128B descriptors, half the modeled DMA time per row), soaking
otherwise-idle Pool-engine time to take load off the saturated DMA engines.
DVE multiplies gathered chunks by per-slot weights and does a strided
per-block reduce; PE transposes agg and applies W^T in 4-block groups; the
Activation engine does the PSUM->SBUF copies; outputs are written as
contiguous [64, cols] bf16 (no indirect scatter) and un-permuted on host.
"""

import numpy as np
from contextlib import ExitStack
from ml_dtypes import bfloat16

import concourse.bass as bass
import concourse.bacc as bacc
import concourse.mybir as mybir
import concourse.tile as tile
from concourse import library_config
from concourse.bass import IndirectOffsetOnAxis
from concourse.bass_utils import run_bass_kernel_spmd

P = 128
D = 64
NQ = 4          # source quarters
NH = 2          # target halves
QS = 25000      # sources per quarter
HS = 50000      # targets per half
NT_PAD = 50048  # targets padded to block multiple
NBLK = NT_PAD // P  # 391
N_CORES = 8
SC_COLS = 64       # columns per ANT superchunk (mult/msg tile granularity)
GA_COLS = 8        # columns per dma_gather (1024 idx = SWDGE ring capacity)
IND_COLS = 35      # columns routed to the indirect bf16 path
N_QUEUES = 4
GRP = 4

F32 = mybir.dt.float32
BF16 = mybir.dt.bfloat16
I16 = mybir.dt.int16
I32 = mybir.dt.int32


def _make_schedule(d_blocks):
    """Split blocks into indirect/ANT families and build the processing
    schedule: ANT superchunks (tapered near the end) with indirect blocks
    interleaved, ending on a single indirect block for a minimal tail.

    Returns (sched, meta) where sched is a list of ("ant", ids) /
    ("ind", ids) entries and meta maps each block id to
    (family, fam_col_off, glob_col_off, proc_index).
    """
    K = len(d_blocks)
    ind_ids, cols_ind = [], 0
    for j in range(K):
        dj = int(d_blocks[j])
        if dj <= 2 and cols_ind + dj <= IND_COLS:
            ind_ids.append(j)
            cols_ind += dj
        else:
            break
    ant_ids = list(range(len(ind_ids), K))

    total = int(sum(int(d_blocks[j]) for j in ant_ids))
    ant_chunks, i, done = [], 0, 0
    while i < len(ant_ids):
        left = total - done
        cap = SC_COLS if left > 2 * SC_COLS else max(GA_COLS, left // 4)
        ids, cols = [], 0
        while i < len(ant_ids) and cols + int(d_blocks[ant_ids[i]]) <= cap:
            ids.append(ant_ids[i])
            cols += int(d_blocks[ant_ids[i]])
            i += 1
        if not ids:
            ids = [ant_ids[i]]
            cols = int(d_blocks[ant_ids[i]])
            i += 1
        ant_chunks.append(ids)
        done += cols

    sched = []
    tail_ind = ind_ids[-1:]
    rest = ind_ids[:-1]
    nch = max(1, len(ant_chunks))
    for ci, ids in enumerate(ant_chunks):
        sched.append(("ant", ids))
        lo = len(rest) * ci // nch
        hi = len(rest) * (ci + 1) // nch
        for j in rest[lo:hi]:
            sched.append(("ind", [j]))
    if tail_ind:
        sched.append(("ind", tail_ind))

    meta = {}
    fam_off = {"ant": 0, "ind": 0}
    gcol, pidx = 0, 0
    for fam, ids in sched:
        for j in ids:
            meta[j] = (fam, fam_off[fam], gcol, pidx)
            fam_off[fam] += int(d_blocks[j])
            gcol += int(d_blocks[j])
            pidx += 1
    return sched, meta, fam_off["ant"], fam_off["ind"]


def preprocess(source, target, edge_weights):
    """Shared block schedule + per-core gather/weight arrays."""
    source = np.asarray(source).astype(np.int64)
    target = np.asarray(target).astype(np.int64)
    w_all = np.asarray(edge_weights).astype(np.float32)

    cores = []
    for q in range(NQ):
        for h in range(NH):
            m = ((source >= q * QS) & (source < (q + 1) * QS)
                 & (target >= h * HS) & (target < (h + 1) * HS))
            sl = (source[m] - q * QS).astype(np.int64)
            tl = (target[m] - h * HS).astype(np.int64)
            w = w_all[m]
            deg = np.bincount(tl, minlength=NT_PAD)
            perm = np.argsort(deg, kind="stable")
            rank = np.empty(NT_PAD, dtype=np.int64)
            rank[perm] = np.arange(NT_PAD)
            dmax = deg[perm].reshape(NBLK, P).max(axis=1)
            cores.append(dict(sl=sl, tl=tl, w=w, deg=deg, rank=rank, dmax=dmax))

    dmax_sh = np.max([c["dmax"] for c in cores], axis=0)
    nskip = int(np.argmax(dmax_sh > 0))  # leading all-zero blocks
    d_blocks = dmax_sh[nskip:].astype(np.int64)
    K = len(d_blocks)

    sched, meta, S_ant, S_ind = _make_schedule(d_blocks)
    S = S_ant + S_ind
    fam_arr = np.array([0 if meta[j][0] == "ant" else 1 for j in range(K)])
    foff_arr = np.array([meta[j][1] for j in range(K)])
    goff_arr = np.array([meta[j][2] for j in range(K)])
    posn = np.array([meta[j][3] for j in range(K)])

    per_core = []
    for c in cores:
        deg, rank = c["deg"], c["rank"]
        order = np.argsort(c["tl"], kind="stable")
        sl_s, w_s, tl_s = c["sl"][order], c["w"][order], c["tl"][order]
        starts = np.concatenate([[0], np.cumsum(deg)])
        slot = np.arange(len(tl_s)) - starts[tl_s]
        r = rank[tl_s]
        jrel = r // P - nskip
        p = r % P
        fam = fam_arr[jrel]
        fcol = foff_arr[jrel] + slot
        gcol = goff_arr[jrel] + slot

        wgt = np.zeros((P, S), dtype=bfloat16)
        wgt[p, gcol] = w_s.astype(bfloat16)

        am = fam == 0
        idx_list = np.zeros(max(S_ant, 1) * P, dtype=np.int16)
        idx_list[fcol[am] * P + p[am]] = sl_s[am].astype(np.int16)
        idx_arr = np.tile(idx_list.reshape(-1, 16).T, (8, 1))

        idxi = np.zeros((P, max(S_ind, 1)), dtype=np.int32)
        im = ~am
        idxi[p[im], fcol[im]] = sl_s[im].astype(np.int32)

        per_core.append(dict(idx=np.ascontiguousarray(idx_arr), wgt=wgt,
                             idxi=idxi, rank=rank))

    return dict(d_blocks=[int(d) for d in d_blocks], sched=sched,
                meta=meta, S_ant=S_ant, S_ind=S_ind, S=S, nskip=nskip,
                posn=posn, per_core=per_core)


def build_nc(pp, bufs=6):
    d_blocks = pp["d_blocks"]
    sched, meta = pp["sched"], pp["meta"]
    S_ant, S_ind, S = pp["S_ant"], pp["S_ind"], pp["S"]
    K = len(d_blocks)
    ncols = K * P
    max_ant = max((sum(int(d_blocks[j]) for j in ids)
                   for fam, ids in sched if fam == "ant"), default=SC_COLS)
    maxc = max(SC_COLS, max_ant)
    max_ind = max((int(d_blocks[ids[0]]) for fam, ids in sched
                   if fam == "ind"), default=1)

    nc = bacc.Bacc("TRN2", target_bir_lowering=False, debug=False,
                   num_swdge_queues=N_QUEUES)
    xq_t = nc.dram_tensor("xq", [QS, D], F32, kind="ExternalInput")
    xb_t = nc.dram_tensor("xb", [QS, D], BF16, kind="ExternalInput")
    idx_t = nc.dram_tensor("idx", [P, max(S_ant, 1) * 8], I16,
                           kind="ExternalInput")
    idxi_t = nc.dram_tensor("idxi", [P, max(S_ind, 1)], I32,
                            kind="ExternalInput")
    wgt_t = nc.dram_tensor("wgt", [P, S], BF16, kind="ExternalInput")
    wt_t = nc.dram_tensor("wT", [D, D], BF16, kind="ExternalInput")
    eye_t = nc.dram_tensor("eye", [P, P], F32, kind="ExternalInput")
    out_t = nc.dram_tensor("out", [D, ncols], BF16, kind="ExternalOutput")

    with tile.TileContext(nc) as tc, ExitStack() as ctx:
        nc.gpsimd.load_library(library_config.mlp)
        const = ctx.enter_context(tc.tile_pool(name="const", bufs=1))
        gpool = ctx.enter_context(tc.tile_pool(name="gather", bufs=bufs))
        mpool = ctx.enter_context(tc.tile_pool(name="msg", bufs=bufs))
        ipool = ctx.enter_context(tc.tile_pool(name="indg", bufs=4))
        apool = ctx.enter_context(tc.tile_pool(name="agg", bufs=2 * GRP))
        tpool = ctx.enter_context(tc.tile_pool(name="aggT", bufs=3))
        opool = ctx.enter_context(tc.tile_pool(name="osb", bufs=3))
        psumT = ctx.enter_context(tc.tile_pool(name="psumT", bufs=3, space="PSUM"))
        psumM = ctx.enter_context(tc.tile_pool(name="psumM", bufs=3, space="PSUM"))

        idx_sb = const.tile([P, max(S_ant, 1) * 8], I16)
        bounds = [0, GA_COLS] + list(range(128, S_ant, 128)) + [S_ant]
        for i, e in zip(bounds, bounds[1:]):
            if e > i:
                nc.sync.dma_start(out=idx_sb[:, i * 8:e * 8],
                                  in_=idx_t.ap()[:, i * 8:e * 8])
        idxi_sb = const.tile([P, max(S_ind, 1)], I32)
        nc.sync.dma_start(out=idxi_sb[:], in_=idxi_t.ap())
        wgt_sb = const.tile([P, S], BF16)
        nc.sync.dma_start(out=wgt_sb[:], in_=wgt_t.ap())
        wt_sb = const.tile([D, D], BF16)
        nc.sync.dma_start(out=wt_sb[:], in_=wt_t.ap())
        eye_sb = const.tile([P, P], F32)
        nc.sync.dma_start(out=eye_sb[:], in_=eye_t.ap())

        # Prime engines on the upfront loads so later instructions carry at
        # most one sync wait each.
        prime = const.tile([P, 1], BF16)
        nc.vector.tensor_copy(out=prime[:], in_=wgt_sb[:, :1])
        prime_a = const.tile([P, 1], BF16)
        nc.scalar.copy(out=prime_a[:], in_=wgt_sb[:, :1])
        prime_ps = psumT.tile([D, GRP * P], F32, tag="tp")
        nc.tensor.transpose(out=prime_ps[:, :P], in_=eye_sb[:, :D],
                            identity=eye_sb[:])

        group = []          # agg tiles pending transpose+transform
        gcol = 0            # output column of first block in group
        osb_cur = [None, 0, 0]  # tile, fill cols, start col

        def flush_osb():
            tile_, fill, start = osb_cur
            if tile_ is None or fill == 0:
                return
            nc.sync.dma_start(out=out_t.ap()[:, start:start + fill],
                              in_=tile_[:, :fill])
            osb_cur[0], osb_cur[1] = None, 0

        def flush_group():
            nonlocal group, gcol
            n = len(group)
            if n == 0:
                return
            w = n * P
            psT = psumT.tile([D, GRP * P], F32, tag="tp")
            for i, a in enumerate(group):
                nc.tensor.transpose(out=psT[:, i * P:(i + 1) * P], in_=a[:],
                                    identity=eye_sb[:])
            aggT = tpool.tile([D, GRP * P], BF16, tag="aT")
            nc.scalar.copy(out=aggT[:, :w], in_=psT[:, :w])
            ps2 = psumM.tile([D, GRP * P], F32, tag="mm")
            nc.tensor.matmul(out=ps2[:, :w], lhsT=wt_sb[:], rhs=aggT[:, :w],
                             start=True, stop=True)
            if osb_cur[0] is None:
                osb_cur[0] = opool.tile([D, 4 * GRP * P], BF16, tag="o",
                                        name="osb")
                osb_cur[1] = 0
                osb_cur[2] = gcol
            f = osb_cur[1]
            nc.scalar.copy(out=osb_cur[0][:, f:f + w], in_=ps2[:, :w])
            osb_cur[1] = f + w
            if osb_cur[1] == 4 * GRP * P:
                flush_osb()
            group = []
            gcol += w

        def do_block(msg_ap, off_in_tile, dj, gcol_blk):
            agg = apool.tile([P, D], F32, tag="agg")
            nc.vector.tensor_reduce(
                out=agg[:],
                in_=msg_ap[:, off_in_tile * D:(off_in_tile + dj) * D].rearrange(
                    "p (d o) -> p o d", o=D),
                axis=mybir.AxisListType.X,
                op=mybir.AluOpType.add,
            )
            group.append(agg)
            if len(group) == GRP:
                flush_group()

        qn = 1
        n_ent = len(sched)
        for ei, (fam, ids) in enumerate(sched):
            if ei == n_ent - 1:
                flush_group()  # keep the final group down to the last block
            if fam == "ant":
                ccols = sum(int(d_blocks[j]) for j in ids)
                fcol0 = meta[ids[0]][1]
                gcol0 = meta[ids[0]][2]
                g = gpool.tile([P, maxc * D], F32, tag="g")
                msg = mpool.tile([P, maxc * D], BF16, tag="m")
                for c0 in range(0, ccols, GA_COLS):
                    cc = min(GA_COLS, ccols - c0)
                    nc.gpsimd.dma_gather(
                        out_ap=g[:, c0 * D:(c0 + cc) * D].rearrange(
                            "p (c o) -> p c o", o=D),
                        in_ap=xq_t.ap(),
                        idxs_ap=idx_sb[:, (fcol0 + c0) * 8:
                                       (fcol0 + c0 + cc) * 8],
                        num_idxs=cc * P,
                        num_idxs_reg=cc * P,
                        elem_size=D,
                        queue_num=qn,
                    )
                    qn = 1 + (qn % (N_QUEUES - 1))
                    nc.vector.tensor_tensor(
                        out=msg[:, c0 * D:(c0 + cc) * D].rearrange(
                            "p (c o) -> p c o", o=D),
                        in0=g[:, c0 * D:(c0 + cc) * D].rearrange(
                            "p (c o) -> p c o", o=D),
                        in1=wgt_sb[:, gcol0 + c0:gcol0 + c0 + cc].to_broadcast(
                            [P, cc, D]),
                        op=mybir.AluOpType.mult,
                    )
                o = 0
                for j in ids:
                    dj = int(d_blocks[j])
                    do_block(msg, o, dj, meta[j][2])
                    o += dj
            else:
                j = ids[0]
                dj = int(d_blocks[j])
                fcol0, gcol0 = meta[j][1], meta[j][2]
                g2 = ipool.tile([P, max_ind * D], BF16, tag="g2")
                for dd in range(dj):
                    nc.gpsimd.indirect_dma_start(
                        out=g2[:, dd * D:(dd + 1) * D],
                        out_offset=None,
                        in_=xb_t.ap(),
                        in_offset=IndirectOffsetOnAxis(
                            ap=idxi_sb[:, fcol0 + dd:fcol0 + dd + 1], axis=0),
                    )
                msg2 = ipool.tile([P, max_ind * D], BF16, tag="m2")
                nc.vector.tensor_tensor(
                    out=msg2[:, :dj * D].rearrange("p (c o) -> p c o", o=D),
                    in0=g2[:, :dj * D].rearrange("p (c o) -> p c o", o=D),
                    in1=wgt_sb[:, gcol0:gcol0 + dj].to_broadcast([P, dj, D]),
                    op=mybir.AluOpType.mult,
                )
                do_block(msg2, 0, dj, gcol0)
        flush_group()
        flush_osb()
    nc.compile()
    return nc


def run_gcn(x, W, edge_weights, source, target, num_nodes, trace=False, bufs=6):
    """Full-input host entry: preprocess, build, run on 8 cores, assemble."""
    assert int(num_nodes) == NQ * QS
    x = np.ascontiguousarray(np.asarray(x), dtype=np.float32)
    pp = preprocess(source, target, edge_weights)
    nc = build_nc(pp, bufs=bufs)

    wt_np = np.ascontiguousarray(np.asarray(W).T).astype(bfloat16)
    eye_np = np.eye(P, dtype=np.float32)
    in_maps = []
    for q in range(NQ):
        for h in range(NH):
            pc = pp["per_core"][q * NH + h]
            xq = x[q * QS:(q + 1) * QS]
            in_maps.append({
                "xq": xq, "xb": xq.astype(bfloat16),
                "idx": pc["idx"], "idxi": pc["idxi"], "wgt": pc["wgt"],
                "wT": wt_np, "eye": eye_np,
            })
    res = run_bass_kernel_spmd(nc, in_maps, core_ids=list(range(N_CORES)),
                               trace=trace)

    nskip, posn = pp["nskip"], pp["posn"]
    # map sorted-rank -> device output column (-1 for skipped zero blocks)
    r = np.arange(NT_PAD)
    jrel = r // P - nskip
    colmap = np.where(jrel >= 0, posn[np.maximum(jrel, 0)] * P + r % P, -1)
    delta = np.zeros((NQ * QS, D), dtype=np.float32)
    tids = np.arange(HS)
    for q in range(NQ):
        for h in range(NH):
            k = q * NH + h
            out = np.asarray(res.results[k]["out"]).astype(np.float32)
            out_ext = np.concatenate(
                [out, np.zeros((D, 1), dtype=np.float32)], axis=1)
            rank = pp["per_core"][k]["rank"]
            delta[h * HS:(h + 1) * HS] += out_ext[:, colmap[rank[tids]]].T
    return delta, res


def kernel(**inputs) -> np.ndarray:
    """Harness entry: full unsharded inputs -> full (num_nodes, 64) output."""
    out, _ = run_gcn(
        np.asarray(inputs["x"]),
        np.asarray(inputs["W"]),
        np.asarray(inputs["edge_weights"]),
        np.asarray(inputs["source"]),
        np.asarray(inputs["target"]),
        int(inputs["num_nodes"]),
        trace=False,
    )
    return out


# revision 29
# speedup vs baseline: 13.1139x; 1.1610x over previous
"""GCN message-passing Bass kernel for TRN2 (8 cores).

Math: delta = segment_sum(w_e * x[src_e]) @ W^T  (linearity: transform after
aggregate).

Sharding: 4 source-quarters x 2 target-halves = 8 cores. Each core handles
edges whose source lies in its 25k-row quarter of x and whose target lies in
its 50k half; it produces a partial (transposed) delta for its half. The host
sums the 4 quarter-partials per half (the all-reduce/unshard step) -- valid
because delta = (sum_q agg_q) @ W^T is linear.

Per core, targets are degree-sorted (per-core degrees) into 128-target
blocks; block j has d_j padded edge slots. All slots of many blocks are
fetched by ONE ANT dma_gather (int16 indices into the 25k-row quarter --
this is why the source dimension is quartered), amortizing the ~1us
SWDGE descriptor-generation cost that dominated the per-slot indirect-DMA
baseline. DVE multiplies each chunk by per-slot weights and does a strided
per-block reduce; PE transposes agg and applies W^T in 4-block groups; the
Activation engine does the PSUM->SBUF copies; outputs are written as
contiguous [64, cols] bf16 (no indirect scatter) and un-permuted on host.
"""

import numpy as np
from contextlib import ExitStack
from ml_dtypes import bfloat16

import concourse.bass as bass
import concourse.bacc as bacc
import concourse.mybir as mybir
import concourse.tile as tile
from concourse import library_config
from concourse.bass_utils import run_bass_kernel_spmd

P = 128
D = 64
NQ = 4          # source quarters
NH = 2          # target halves
QS = 25000      # sources per quarter
HS = 50000      # targets per half
NT_PAD = 50048  # targets padded to block multiple
NBLK = NT_PAD // P  # 391
N_CORES = 8
SC_COLS = 64       # columns per superchunk (mult granularity / msg tile size)
GA_COLS = 8        # columns per dma_gather (1024 idx = SWDGE ring capacity)
N_QUEUES = 4
GRP = 4

F32 = mybir.dt.float32
BF16 = mybir.dt.bfloat16
I16 = mybir.dt.int16


def preprocess(source, target, edge_weights):
    """Shared block schedule + per-core gather/weight arrays.

    Returns dict with d_sched (shared per-block slot counts, skipped leading
    zero blocks removed), nskip, S, chunks, and per-core idx/wgt arrays and
    rank (sorted position of each local target).
    """
    source = np.asarray(source).astype(np.int64)
    target = np.asarray(target).astype(np.int64)
    w_all = np.asarray(edge_weights).astype(np.float32)

    cores = []
    for q in range(NQ):
        for h in range(NH):
            m = ((source >= q * QS) & (source < (q + 1) * QS)
                 & (target >= h * HS) & (target < (h + 1) * HS))
            sl = (source[m] - q * QS).astype(np.int64)
            tl = (target[m] - h * HS).astype(np.int64)
            w = w_all[m]
            deg = np.bincount(tl, minlength=NT_PAD)
            perm = np.argsort(deg, kind="stable")
            rank = np.empty(NT_PAD, dtype=np.int64)
            rank[perm] = np.arange(NT_PAD)
            dmax = deg[perm].reshape(NBLK, P).max(axis=1)
            cores.append(dict(sl=sl, tl=tl, w=w, deg=deg, rank=rank, dmax=dmax))

    dmax_sh = np.max([c["dmax"] for c in cores], axis=0)
    nskip = int(np.argmax(dmax_sh > 0))  # leading all-zero blocks
    d_blocks = dmax_sh[nskip:].astype(np.int64)
    K = len(d_blocks)
    # processing order: rotate the smallest (first) block to the end so the
    # pipeline tail (last superchunk's DVE chain) is minimal
    proc = np.concatenate([np.arange(1, K), [0]])
    posn = np.empty(K, dtype=np.int64)
    posn[proc] = np.arange(K)
    d_sched = d_blocks[proc]
    offs_p = np.concatenate([[0], np.cumsum(d_sched)])
    S = int(offs_p[-1])

    # superchunks: consecutive blocks with total cols <= cap; the cap tapers
    # near the end (and the rotated-in small block stays alone) so the
    # pipeline tail -- DVE work trailing the last gather -- is minimal
    total = int(np.sum(d_sched[:K - 1]))
    chunks = []  # (first_block_rel, n_blocks, n_cols)
    j, done = 0, 0
    while j < K - 1:
        left = total - done
        cap = SC_COLS if left > 2 * SC_COLS else max(GA_COLS, left // 4)
        j0, cols = j, 0
        while j < K - 1 and cols + d_sched[j] <= cap:
            cols += int(d_sched[j])
            j += 1
        if j == j0:  # single block exceeds the tapered cap
            cols = int(d_sched[j])
            j += 1
        chunks.append((j0, j - j0, cols))
        done += cols
    chunks.append((K - 1, 1, int(d_sched[K - 1])))

    per_core = []
    for c in cores:
        deg, rank = c["deg"], c["rank"]
        # CSR by local target
        order = np.argsort(c["tl"], kind="stable")
        sl_s, w_s, tl_s = c["sl"][order], c["w"][order], c["tl"][order]
        starts = np.concatenate([[0], np.cumsum(deg)])
        slot = np.arange(len(tl_s)) - starts[tl_s]
        r = rank[tl_s]
        jabs = r // P
        p = r % P
        col = offs_p[posn[jabs - nskip]] + slot  # global schedule column
        pos = col * P + p

        idx_list = np.zeros(S * P, dtype=np.int16)
        idx_list[pos] = sl_s.astype(np.int16)
        # index i lives at [16*g + i%16, i//16] for all 8 gpsimd stripes
        idx_arr = np.tile(idx_list.reshape(S * 8, 16).T, (8, 1))
        wgt = np.zeros((P, S), dtype=bfloat16)
        wgt[p, col] = w_s.astype(bfloat16)
        per_core.append(dict(idx=np.ascontiguousarray(idx_arr), wgt=wgt,
                             rank=rank))

    return dict(d_sched=[int(d) for d in d_sched], nskip=nskip, S=S,
                chunks=chunks, per_core=per_core, posn=posn)


def build_nc(d_sched, chunks, S, bufs=2):
    nc = bacc.Bacc("TRN2", target_bir_lowering=False, debug=False,
                   num_swdge_queues=N_QUEUES)
    ncols = len(d_sched) * P
    maxc = max(SC_COLS, max(c[2] for c in chunks))  # tile capacity guard
    xq_t = nc.dram_tensor("xq", [QS, D], F32, kind="ExternalInput")
    idx_t = nc.dram_tensor("idx", [P, S * 8], I16, kind="ExternalInput")
    wgt_t = nc.dram_tensor("wgt", [P, S], BF16, kind="ExternalInput")
    wt_t = nc.dram_tensor("wT", [D, D], BF16, kind="ExternalInput")
    eye_t = nc.dram_tensor("eye", [P, P], F32, kind="ExternalInput")
    out_t = nc.dram_tensor("out", [D, ncols], BF16, kind="ExternalOutput")

    with tile.TileContext(nc) as tc, ExitStack() as ctx:
        nc.gpsimd.load_library(library_config.mlp)
        const = ctx.enter_context(tc.tile_pool(name="const", bufs=1))
        gpool = ctx.enter_context(tc.tile_pool(name="gather", bufs=bufs))
        mpool = ctx.enter_context(tc.tile_pool(name="msg", bufs=bufs))
        apool = ctx.enter_context(tc.tile_pool(name="agg", bufs=2 * GRP))
        tpool = ctx.enter_context(tc.tile_pool(name="aggT", bufs=3))
        opool = ctx.enter_context(tc.tile_pool(name="osb", bufs=3))
        psumT = ctx.enter_context(tc.tile_pool(name="psumT", bufs=3, space="PSUM"))
        psumM = ctx.enter_context(tc.tile_pool(name="psumM", bufs=3, space="PSUM"))

        idx_sb = const.tile([P, S * 8], I16)
        bounds = [0, GA_COLS] + list(range(128, S, 128)) + [S]
        for i, e in zip(bounds, bounds[1:]):
            if e > i:
                nc.sync.dma_start(out=idx_sb[:, i * 8:e * 8],
                                  in_=idx_t.ap()[:, i * 8:e * 8])
        wgt_sb = const.tile([P, S], BF16)
        nc.sync.dma_start(out=wgt_sb[:], in_=wgt_t.ap())
        wt_sb = const.tile([D, D], BF16)
        nc.sync.dma_start(out=wt_sb[:], in_=wt_t.ap())
        eye_sb = const.tile([P, P], F32)
        nc.sync.dma_start(out=eye_sb[:], in_=eye_t.ap())

        # Prime engines on the upfront loads so later instructions carry at
        # most one sync wait each.
        prime = const.tile([P, 1], BF16)
        nc.vector.tensor_copy(out=prime[:], in_=wgt_sb[:, :1])
        prime_a = const.tile([P, 1], BF16)
        nc.scalar.copy(out=prime_a[:], in_=wgt_sb[:, :1])
        prime_ps = psumT.tile([D, GRP * P], F32, tag="tp")
        nc.tensor.transpose(out=prime_ps[:, :P], in_=eye_sb[:, :D],
                            identity=eye_sb[:])

        group = []          # agg tiles pending transpose+transform
        gcol = 0            # output column of first block in group
        osb_cur = [None, 0, 0]  # tile, fill cols, start col

        def flush_osb():
            tile_, fill, start = osb_cur
            if tile_ is None or fill == 0:
                return
            nc.sync.dma_start(out=out_t.ap()[:, start:start + fill],
                              in_=tile_[:, :fill])
            osb_cur[0], osb_cur[1] = None, 0

        def flush_group():
            nonlocal group, gcol
            n = len(group)
            if n == 0:
                return
            w = n * P
            psT = psumT.tile([D, GRP * P], F32, tag="tp")
            for i, a in enumerate(group):
                nc.tensor.transpose(out=psT[:, i * P:(i + 1) * P], in_=a[:],
                                    identity=eye_sb[:])
            aggT = tpool.tile([D, GRP * P], BF16, tag="aT")
            nc.scalar.copy(out=aggT[:, :w], in_=psT[:, :w])
            ps2 = psumM.tile([D, GRP * P], F32, tag="mm")
            nc.tensor.matmul(out=ps2[:, :w], lhsT=wt_sb[:], rhs=aggT[:, :w],
                             start=True, stop=True)
            if osb_cur[0] is None:
                osb_cur[0] = opool.tile([D, 4 * GRP * P], BF16, tag="o",
                                        name="osb")
                osb_cur[1] = 0
                osb_cur[2] = gcol
            f = osb_cur[1]
            nc.scalar.copy(out=osb_cur[0][:, f:f + w], in_=ps2[:, :w])
            osb_cur[1] = f + w
            if osb_cur[1] == 4 * GRP * P:
                flush_osb()
            group = []
            gcol += w

        col = 0
        qn = 0
        for ci, (j0, nblks, ccols) in enumerate(chunks):
            if ci == len(chunks) - 1:
                flush_group()  # keep the final group down to the last block
            g = gpool.tile([P, maxc * D], F32, tag="g")
            msg = mpool.tile([P, maxc * D], BF16, tag="m")
            for c0 in range(0, ccols, GA_COLS):
                cc = min(GA_COLS, ccols - c0)
                nc.gpsimd.dma_gather(
                    out_ap=g[:, c0 * D:(c0 + cc) * D].rearrange(
                        "p (c o) -> p c o", o=D),
                    in_ap=xq_t.ap(),
                    idxs_ap=idx_sb[:, (col + c0) * 8:(col + c0 + cc) * 8],
                    num_idxs=cc * P,
                    num_idxs_reg=cc * P,
                    elem_size=D,
                    queue_num=qn,
                )
                qn = (qn + 1) % N_QUEUES
                nc.vector.tensor_tensor(
                    out=msg[:, c0 * D:(c0 + cc) * D].rearrange(
                        "p (c o) -> p c o", o=D),
                    in0=g[:, c0 * D:(c0 + cc) * D].rearrange(
                        "p (c o) -> p c o", o=D),
                    in1=wgt_sb[:, col + c0:col + c0 + cc].to_broadcast(
                        [P, cc, D]),
                    op=mybir.AluOpType.mult,
                )
            o = 0
            for jj in range(nblks):
                dj = d_sched[j0 + jj]
                agg = apool.tile([P, D], F32, tag="agg")
                nc.vector.tensor_reduce(
                    out=agg[:],
                    in_=msg[:, o * D:(o + dj) * D].rearrange(
                        "p (d o) -> p o d", o=D),
                    axis=mybir.AxisListType.X,
                    op=mybir.AluOpType.add,
                )
                group.append(agg)
                if len(group) == GRP:
                    flush_group()
                o += dj
            col += ccols
        flush_group()
        flush_osb()
    nc.compile()
    return nc


def run_gcn(x, W, edge_weights, source, target, num_nodes, trace=False, bufs=6):
    """Full-input host entry: preprocess, build, run on 8 cores, assemble."""
    assert int(num_nodes) == NQ * QS
    x = np.ascontiguousarray(np.asarray(x), dtype=np.float32)
    pp = preprocess(source, target, edge_weights)
    nc = build_nc(pp["d_sched"], pp["chunks"], pp["S"], bufs=bufs)

    wt_np = np.ascontiguousarray(np.asarray(W).T).astype(bfloat16)
    eye_np = np.eye(P, dtype=np.float32)
    in_maps = []
    for q in range(NQ):
        for h in range(NH):
            pc = pp["per_core"][q * NH + h]
            in_maps.append({
                "xq": x[q * QS:(q + 1) * QS],
                "idx": pc["idx"], "wgt": pc["wgt"],
                "wT": wt_np, "eye": eye_np,
            })
    res = run_bass_kernel_spmd(nc, in_maps, core_ids=list(range(N_CORES)),
                               trace=trace)

    nskip, posn = pp["nskip"], pp["posn"]
    # map sorted-rank -> device output column (-1 for skipped zero blocks)
    r = np.arange(NT_PAD)
    jrel = r // P - nskip
    colmap = np.where(jrel >= 0, posn[np.maximum(jrel, 0)] * P + r % P, -1)
    delta = np.zeros((NQ * QS, D), dtype=np.float32)
    tids = np.arange(HS)
    for q in range(NQ):
        for h in range(NH):
            k = q * NH + h
            out = np.asarray(res.results[k]["out"]).astype(np.float32)
            out_ext = np.concatenate(
                [out, np.zeros((D, 1), dtype=np.float32)], axis=1)
            rank = pp["per_core"][k]["rank"]
            delta[h * HS:(h + 1) * HS] += out_ext[:, colmap[rank[tids]]].T
    return delta, res


def kernel(**inputs) -> np.ndarray:
    """Harness entry: full unsharded inputs -> full (num_nodes, 64) output."""
    out, _ = run_gcn(
        np.asarray(inputs["x"]),
        np.asarray(inputs["W"]),
        np.asarray(inputs["edge_weights"]),
        np.asarray(inputs["source"]),
        np.asarray(inputs["target"]),
        int(inputs["num_nodes"]),
        trace=False,
    )
    return out
